# revision 28
# baseline (speedup 1.0000x reference)
"""Trainium2 Bass kernel for GroundwaterModel Jacobi pseudo-timestepping.

100 Jacobi steps of -div(exp(u) grad p) = f on a [1024,1024] grid, with the
symmetrizing substitution q = sqrt(D) p (D = Jacobi diagonal), so the update is

  q'[i,j] = bxu[i,j] q[i+1,j] + bxd[i,j] q[i-1,j]
          + byu[i,j] q[i,j+1] + byd[i,j] q[i,j-1] + c[i,j]

Sharding: columns across 8 cores (128 cols each), rows chunked into the
partition dim: partition p holds rows 8p..8p+7, free dim = 8 row-windows of
CW = 128 + 2*GW columns (GW ghost columns each side).  With this layout both
x- and y-shifts are free-dim offsets applied by PE identity matmuls into PSUM
(plus 4 tiny cross-partition shift matmuls for row-chunk boundaries), the DVE
only computes the 4 coefficient products, and the ACT engine evicts PSUM.
Ghost columns are updated redundantly each step, so the halo AllGather runs
only every GW steps and stays off the per-step critical path.  fp16 datapath
(PSUM accumulation in fp32).
"""

import numpy as np

N = 1024
NCORES = 8
RP = 8               # rows per partition chunk
PART = 128           # partitions
MC = 128             # main cols per core
GW = 12              # ghost width == exchange period
CW = MC + 2 * GW     # stored cols per row-window (152)
F = RP * CW          # free size (1216)
TS = 100
BANKS = [(0, 512), (512, 1024), (1024, F)]

_cached = {}


def _coeff_full(u, f):
    """Full-grid float64 coefficient arrays."""
    u = np.asarray(u, np.float64)
    f = np.asarray(f, np.float64)
    h = 1.0 / (N - 1)
    eu = np.exp(u)
    eu_xm = np.concatenate([eu[:1, :], eu[:-1, :]], axis=0)
    eu_ym = np.concatenate([eu[:, :1], eu[:, :-1]], axis=1)
    den = 2.0 * eu + eu_xm + eu_ym
    s = np.sqrt(den)
    rs = 1.0 / s
    s_xp = np.concatenate([s[1:, :], s[-1:, :]], axis=0)
    s_xm = np.concatenate([s[:1, :], s[:-1, :]], axis=0)
    s_yp = np.concatenate([s[:, 1:], s[:, -1:]], axis=1)
    s_ym = np.concatenate([s[:, :1], s[:, :-1]], axis=1)
    # edge-replicated s_xp/s_xm make rows 0/1023 the Neumann self-terms
    bxu = eu / (s * s_xp)
    bxd = eu_xm / (s * s_xm)
    byu = eu / (s * s_yp)
    byd = eu_ym / (s * s_ym)
    cp = h * h * f * rs
    # Dirichlet columns: dead cells holding q = s*bc from step 1 on.
    # Zeroing follows the product-shift consumption pattern:
    #   col j's y-up   arrives via byd[j+1] (tensor C1 = byd*q read at +1)
    #   col j's y-down arrives via byu[j-1] (tensor D1 = byu*q read at -1)
    #   col j's x-up/down arrive via bxd[.,j]/bxu[.,j] (partition shifts)
    xs = np.arange(N, dtype=np.float64) * h
    cp[:, 0] = s[:, 0] * xs
    cp[:, N - 1] = s[:, N - 1] * (1.0 - xs)
    bxu[:, 0] = bxd[:, 0] = 0.0       # kill col 0 x-terms
    bxu[:, N - 1] = bxd[:, N - 1] = 0.0
    byd[:, 0] = 0.0                   # feeds only a dead ghost; keep clean
    byd[:, 1] = 0.0                   # col 0 must not receive y-up
    byu[:, N - 2] = 0.0               # col N-1 must not receive y-down
    byu[:, N - 1] = 0.0               # feeds only a dead ghost
    # byu[:, 0] and byd[:, N-1] KEEP true values: they supply col 1's y-down
    # and col N-2's y-up respectively.
    return bxu, bxd, byu, byd, cp, rs


def _tile_of(full, c, dtype=np.float16):
    """[1024,1024] full-grid array -> per-core [128, RP, CW] tile."""
    out = np.zeros((PART, RP, CW), np.float64)
    jj = np.arange(CW) + MC * c - GW
    valid = (jj >= 0) & (jj < N)
    out[:, :, valid] = full.reshape(PART, RP, N)[:, :, jj[valid]]
    return out.astype(dtype)


def _host_inputs(u, f):
    bxu, bxd, byu, byd, cp, rs = _coeff_full(u, f)

    ident = np.eye(PART, dtype=np.float16)
    wup = np.zeros((PART, PART), np.float16)   # out p <- in p+1
    for p in range(PART - 1):
        wup[p + 1, p] = 1.0
    wdn = np.zeros((PART, PART), np.float16)   # out p <- in p-1
    for p in range(1, PART):
        wdn[p - 1, p] = 1.0
    wc0 = np.zeros((PART, PART), np.float16)
    wc0[0, 0] = 1.0
    wc127 = np.zeros((PART, PART), np.float16)
    wc127[127, 127] = 1.0

    in_maps = []
    for c in range(NCORES):
        m = np.zeros((PART, NCORES, RP * 2 * GW), np.float16)
        sel = np.zeros((NCORES, RP, 2 * GW), np.float16)
        if c > 0:
            sel[c - 1, :, GW:2 * GW] = 1.0   # left neighbor's right-send
        if c < NCORES - 1:
            sel[c + 1, :, 0:GW] = 1.0        # right neighbor's left-send
        m[:, :, :] = sel.reshape(NCORES, RP * 2 * GW)[None, :, :]
        in_maps.append({
            "bxdT": _tile_of(bxd, c),
            "bxuT": _tile_of(bxu, c),
            "bydT": _tile_of(byd, c),
            "byuT": _tile_of(byu, c),
            "cpT": _tile_of(cp, c),
            "rsT": _tile_of(rs, c, np.float32)[:, :, GW:GW + MC].copy(),
            "mask": m,
            "wI": ident, "wUp": wup, "wDn": wdn,
            "wC0": wc0, "wC127": wc127,
        })
    return in_maps


def _build():
    import concourse.bass as bass
    import concourse.bacc as bacc
    import concourse.mybir as mybir
    from concourse.tile import TileContext

    f32 = mybir.dt.float32
    f16 = mybir.dt.float16
    AF = mybir.ActivationFunctionType
    OP = mybir.AluOpType
    SG = RP * 2 * GW                     # send payload elems per partition

    nc = bacc.Bacc("TRN2", target_bir_lowering=False, debug=False,
                   num_devices=NCORES)
    dp = nc.declare_dram_parameter
    bxd_d = dp("bxdT", [PART, RP, CW], f16, isOutput=False)
    bxu_d = dp("bxuT", [PART, RP, CW], f16, isOutput=False)
    byd_d = dp("bydT", [PART, RP, CW], f16, isOutput=False)
    byu_d = dp("byuT", [PART, RP, CW], f16, isOutput=False)
    cp_d = dp("cpT", [PART, RP, CW], f16, isOutput=False)
    rs_d = dp("rsT", [PART, RP, MC], f32, isOutput=False)
    mask_d = dp("mask", [PART, NCORES, SG], f16, isOutput=False)
    w_ds = {nm: dp(nm, [PART, PART], f16, isOutput=False)
            for nm in ("wI", "wUp", "wDn", "wC0", "wC127")}
    pout_d = dp("pout", [PART, RP * MC], f32, isOutput=True)

    with TileContext(nc) as tc:
        with (
            tc.tile_pool(name="coef", bufs=1) as coef,
            tc.tile_pool(name="work", bufs=2) as work,
            tc.tile_pool(name="qp", bufs=2, space="PSUM") as qp,
            tc.tile_pool(name="dramp", bufs=2, space="DRAM") as dramp,
        ):
            bxdT = coef.tile([PART, RP, CW], f16, name="bxdT_t")
            bxuT = coef.tile([PART, RP, CW], f16, name="bxuT_t")
            bydT = coef.tile([PART, RP, CW], f16, name="bydT_t")
            byuT = coef.tile([PART, RP, CW], f16, name="byuT_t")
            cpT = coef.tile([PART, RP, CW], f16, name="cpT_t")
            rsT = coef.tile([PART, RP, MC], f32, name="rsT_t")
            mask = coef.tile([PART, NCORES, SG], f16, name="mask_t")
            ws = {nm: coef.tile([PART, PART], f16, name=f"{nm}_t")
                  for nm in w_ds}
            nc.sync.dma_start(out=bxdT[:, :, :], in_=bxd_d[:, :, :])
            nc.sync.dma_start(out=bxuT[:, :, :], in_=bxu_d[:, :, :])
            nc.sync.dma_start(out=bydT[:, :, :], in_=byd_d[:, :, :])
            nc.sync.dma_start(out=byuT[:, :, :], in_=byu_d[:, :, :])
            nc.sync.dma_start(out=cpT[:, :, :], in_=cp_d[:, :, :])
            nc.sync.dma_start(out=rsT[:, :, :], in_=rs_d[:, :, :])
            nc.sync.dma_start(out=mask[:, :, :], in_=mask_d[:, :, :])
            for nm, d in w_ds.items():
                nc.sync.dma_start(out=ws[nm][:, :], in_=d[:, :])

            cpF = cpT.rearrange("p r c -> p (r c)")
            V = nc.vector
            mm = nc.tensor.matmul

            q = work.tile([PART, RP, CW], f16, tag="q", name="q_0")
            V.memset(q[:, :, :], 0.0)

            for t in range(1, TS + 1):
                A = work.tile([PART, RP, CW], f16, tag="A", name=f"A_{t}")
                B = work.tile([PART, RP, CW], f16, tag="B", name=f"B_{t}")
                C1 = work.tile([PART, RP, CW], f16, tag="C1", name=f"C1_{t}")
                D1 = work.tile([PART, RP, CW], f16, tag="D1", name=f"D1_{t}")
                V.tensor_mul(A[:, :, :], bxdT[:, :, :], q[:, :, :])
                V.tensor_mul(B[:, :, :], bxuT[:, :, :], q[:, :, :])
                V.tensor_mul(C1[:, :, :], bydT[:, :, :], q[:, :, :])
                V.tensor_mul(D1[:, :, :], byuT[:, :, :], q[:, :, :])
                Af = A.rearrange("p r c -> p (r c)")
                Bf = B.rearrange("p r c -> p (r c)")
                Cf = C1.rearrange("p r c -> p (r c)")
                Df = D1.rearrange("p r c -> p (r c)")

                ps = qp.tile([PART, 1536], f32, tag="ps", name=f"ps_{t}")
                # Emit matmuls in global dependency order (PE executes its
                # queue in order; grouping by bank would stall PE on the last
                # product once per bank).  Per-bank accumulation groups:
                # cp opens (start=True), y-down closes (stop=True).
                for lo, hi in BANKS:
                    mm(ps[:, lo:hi], ws["wI"], cpF[:, lo:hi],
                       start=True, stop=False)
                for lo, hi in BANKS:
                    # x-up: out x gets A[x+CW], valid out < 7*CW
                    a, b = lo, min(hi, 7 * CW)
                    if a < b:
                        mm(ps[:, a:b], ws["wI"], Af[:, a + CW:b + CW],
                           start=False, stop=False)
                for lo, hi in BANKS:
                    # x-down: out x gets B[x-CW], valid out >= CW
                    a, b = max(lo, CW), hi
                    if a < b:
                        mm(ps[:, a:b], ws["wI"], Bf[:, a - CW:b - CW],
                           start=False, stop=False)
                # cross-partition row couplings (need A, B only)
                mm(ps[:, 0:CW], ws["wDn"], Bf[:, 7 * CW:F],
                   start=False, stop=False)
                mm(ps[:, 0:CW], ws["wC0"], Af[:, 0:CW],
                   start=False, stop=False)
                mm(ps[:, 7 * CW:F], ws["wUp"], Af[:, 0:CW],
                   start=False, stop=False)
                mm(ps[:, 7 * CW:F], ws["wC127"], Bf[:, 7 * CW:F],
                   start=False, stop=False)
                for lo, hi in BANKS:
                    # y-up: out x gets C1[x+1], valid out < F-1
                    a, b = lo, min(hi, F - 1)
                    if a < b:
                        mm(ps[:, a:b], ws["wI"], Cf[:, a + 1:b + 1],
                           start=False, stop=False)
                for lo, hi in BANKS:
                    # y-down closes each bank's accumulation group
                    a, b = max(lo, 1), hi
                    mm(ps[:, a:b], ws["wI"], Df[:, a - 1:b - 1],
                       start=False, stop=True)

                qn = work.tile([PART, RP, CW], f16, tag="q", name=f"q_{t}")
                qnF = qn.rearrange("p r c -> p (r c)")
                # bank-sliced eviction, all on DVE: products queue right
                # behind the evicts on the same engine, avoiding the
                # cross-engine semaphore wait that an ACT evict would add
                for lo, hi in BANKS:
                    V.tensor_copy(qnF[:, lo:hi], ps[:, lo:hi])
                q = qn

                if t % GW == 0 and t < TS:
                    stage = work.tile([PART, RP, 2 * GW], f16, tag="stage",
                                      name=f"stage_{t}")
                    V.tensor_copy(stage[:, :, 0:GW], q[:, :, GW:2 * GW])
                    V.tensor_copy(stage[:, :, GW:2 * GW], q[:, :, MC:MC + GW])
                    bounce = dramp.tile([PART, SG], f16, tag="bounce",
                                        name=f"bounce_{t}")
                    gath = dramp.tile([NCORES, PART, SG], f16, tag="gath",
                                      addr_space="Shared", name=f"gath_{t}")
                    stF = stage.rearrange("p r c -> p (r c)")
                    nc.sync.dma_start(out=bounce[:, :], in_=stF[:, :])
                    nc.gpsimd.collective_compute(
                        "AllGather", OP.bypass,
                        ins=[bounce.opt()], outs=[gath.opt()],
                        replica_groups=[list(range(NCORES))],
                    )
                    GG = work.tile([PART, NCORES, SG], f16, tag="GG",
                                   name=f"GG_{t}")
                    nc.sync.dma_start(out=GG[:, :, :],
                                      in_=gath[:, :, :].transpose([1, 0, 2]))
                    GGm = work.tile([PART, NCORES, SG], f16, tag="GGm",
                                    name=f"GGm_{t}")
                    V.tensor_mul(GGm[:, :, :], GG[:, :, :], mask[:, :, :])
                    T1 = work.tile([PART, 4, SG], f16, tag="T1", name=f"T1_{t}")
                    V.tensor_add(T1[:, :, :], GGm[:, 0:4, :], GGm[:, 4:8, :])
                    T2 = work.tile([PART, 2, SG], f16, tag="T2", name=f"T2_{t}")
                    V.tensor_add(T2[:, :, :], T1[:, 0:2, :], T1[:, 2:4, :])
                    R = work.tile([PART, 1, SG], f16, tag="R", name=f"R_{t}")
                    V.tensor_add(R[:, :, :], T2[:, 0:1, :], T2[:, 1:2, :])
                    Rv = R.rearrange("p a (r c) -> p (a r) c", c=2 * GW)
                    V.tensor_copy(q[:, :, 0:GW], Rv[:, :, GW:2 * GW])
                    V.tensor_copy(q[:, :, MC + GW:CW], Rv[:, :, 0:GW])

            outt = coef.tile([PART, RP, MC], f32, name="outt")
            V.tensor_mul(outt[:, :, :], q[:, :, GW:GW + MC], rsT[:, :, :])
            oF = outt.rearrange("p r c -> p (r c)")
            nc.sync.dma_start(out=pout_d[:, :], in_=oF[:, :])

    nc.finalize()
    return nc


def _get_nc():
    if "nc" not in _cached:
        _cached["nc"] = _build()
    return _cached["nc"]


def kernel(u, f, time_steps):
    from concourse.bass_utils import run_bass_kernel_spmd

    u = np.asarray(u)
    f = np.asarray(f)
    assert int(time_steps) == TS and u.shape == (N, N)
    nc = _get_nc()
    in_maps = _host_inputs(u, f)
    res = run_bass_kernel_spmd(nc, in_maps, list(range(NCORES))).results
    h = 1.0 / (N - 1)
    xs = (np.arange(N, dtype=np.float64) * h).astype(np.float32)
    out = np.empty((N, N), dtype=np.float32)
    for c in range(NCORES):
        blk = res[c]["pout"].reshape(PART, RP, MC).reshape(N, MC)
        out[:, MC * c:MC * (c + 1)] = blk
    out[:, 0] = xs
    out[:, N - 1] = 1.0 - xs
    return out
